# revision 1
# baseline (speedup 1.0000x reference)
"""Bahdanau attention on 8 Trainium2 cores (Bass/Tile), data-parallel over B.

reference (per batch b, all shapes full):
    hp  = hidden[0] @ W_h.T + b_h                    # (B, H)
    ep  = einsum('tbh,gh->btg', enc, W_e) + b_e      # (B, T, H)
    en  = tanh(hp[:, None, :] + ep)                  # (B, T, H)
    sc  = en @ v                                     # (B, T)
    out = softmax(sc, -1)[:, None, :]                # (B, 1, T)

Sharding: B=32 split 4-per-core across 8 cores; W_h/W_e/b/v replicated.

Per-core design (HW-measured ~286us per main-loop pass vs 378us for the
fp32r/PE-transpose baseline this replaces):
  - The PE runs ONLY the ep matmuls in bf16 (64 x 512-row per 512-token
    group; bf16 halves SBUF traffic at the same 1 cycle/row rate) plus one
    512-row ones-matmul per group that reduces the v-weighted energies
    over partitions, landing batch b's scores at PSUM partition 32*b via
    tile_position. Keeping the PE queue free of cross-engine data
    dependencies (no transposes, no score matmuls reading ACT output)
    measured faster than every alternative split.
  - enc tiles are loaded f32 with one DMA per group on the Activation
    HWDGE queue, converted f32->bf16 by a single ACT instruction, and
    transposed to [h_lo, hc, t] layout by the DMA XBAR engine
    (dma_start_transpose) on the SP queue. The two DMA streams must sit on
    separate queues (per-queue transfers serialize on HW), and the XBARs
    are emitted at the end of the group so their semaphore waits never
    block a sequencer mid-group; the prep pipeline runs two groups ahead
    of compute (loads three ahead). NOTE: dma_start_transpose on the
    Activation queue silently corrupts data on HW -- it must be on SP.
  - tanh runs on ACT (PSUM f32 in -> bf16 energy out) with the per-
    partition bias hp^T[:, b] + b_h + b_e, writing gc pairs into one
    [128, 1024] tile; a deep ep PSUM pool (5 bufs) keeps the PE from
    waiting on tanh.
  - the v-weighting + reduction over the 8 g-chunks folds on DVE with
    pre-broadcast v tiles: 4 pair-multiplies + 3 adds + 1 fold per group
    (8 instructions instead of 15 scalar-mul/add pairs).
  - softmax skips the max-subtraction (scores are bounded v.tanh sums, so
    f32 exp is exact-safe) and runs once per pass for all four batches:
    2 ACT table switches per pass instead of 8.
"""

import sys
from contextlib import ExitStack

import numpy as np

try:
    import concourse  # noqa: F401
except ImportError:  # pragma: no cover
    sys.path.insert(0, "/opt/trn_rl_repo")

import concourse.tile as tile
from concourse import bacc, mybir
from concourse.bass import ts
from concourse.bass_utils import run_bass_kernel_spmd
from concourse.masks import make_identity

H = 1024
T = 2048
B = 32
NCORES = 8
BC = B // NCORES          # batches per core
HC = H // 128             # h chunks
GC = H // 128             # g chunks
TOK = 512                 # tokens per group (one batch each)
SUB = TOK // 128          # 128-token subtiles per group
NGRP_PER_B = T // TOK
NGRP = BC * NGRP_PER_B

F32 = mybir.dt.float32
BF16 = mybir.dt.bfloat16
AF = mybir.ActivationFunctionType
AX = mybir.AxisListType
ALU = mybir.AluOpType


def build_kernel_nc(reps=1):
    nc = bacc.Bacc(
        "TRN2",
        target_bir_lowering=False,
        debug=False,
        enable_asserts=False,
        num_devices=NCORES,
    )
    enc = nc.dram_tensor("enc", [T, BC, H], F32, kind="ExternalInput").ap()
    hid = nc.dram_tensor("hid", [BC, H], F32, kind="ExternalInput").ap()
    w_e = nc.dram_tensor("W_e", [H, H], F32, kind="ExternalInput").ap()
    w_h = nc.dram_tensor("W_h", [H, H], F32, kind="ExternalInput").ap()
    b_h = nc.dram_tensor("b_h", [H], F32, kind="ExternalInput").ap()
    b_e = nc.dram_tensor("b_e", [H], F32, kind="ExternalInput").ap()
    v = nc.dram_tensor("v", [H], F32, kind="ExternalInput").ap()
    out = nc.dram_tensor("out", [BC, T], F32, kind="ExternalOutput").ap()

    with tile.TileContext(nc) as tc:
        _kernel_body(tc, enc, hid, w_e, w_h, b_h, b_e, v, out, reps=reps)
    nc.compile()
    return nc


def _kernel_body(tc, enc, hid, w_e, w_h, b_h, b_e, v, out, reps=1):
    nc = tc.nc
    with ExitStack() as ctx:
        singles = ctx.enter_context(tc.tile_pool(name="singles", bufs=1))
        enc_pool = ctx.enter_context(tc.tile_pool(name="enc_nat", bufs=3))
        encbf_pool = ctx.enter_context(tc.tile_pool(name="enc_bf", bufs=2))
        encT_pool = ctx.enter_context(tc.tile_pool(name="encT", bufs=3))
        energy_pool = ctx.enter_context(tc.tile_pool(name="energy", bufs=2))
        z_pool = ctx.enter_context(tc.tile_pool(name="zacc", bufs=2))
        tmp_pool = ctx.enter_context(tc.tile_pool(name="ztmp", bufs=2))
        ep_pool = ctx.enter_context(tc.tile_pool(name="epps", bufs=5, space="PSUM"))
        sc_pool = ctx.enter_context(tc.tile_pool(name="scps", bufs=2, space="PSUM"))

        # ---- persistent SBUF tensors -------------------------------------
        WeT = singles.tile([128, HC, H], BF16)     # WeT[h, hc, g] = W_e[g, 128*hc+h]
        bias_all = singles.tile([128, GC, BC], F32)  # hp^T + b_h + b_e
        v_sb = singles.tile([128, GC], F32)        # v[gc*128+p] at [p, gc]
        ones_bf = singles.tile([128, 1], BF16)
        bsum = singles.tile([128, GC], F32)        # (b_h + b_e) chunked
        # batch b's scores live on partition 32*b so per-batch softmax can
        # run as soon as that batch's groups finish
        scores = singles.tile([128, T], F32)
        probs = singles.tile([128, T], F32)
        sums = singles.tile([128, 1], F32)
        rsum = singles.tile([128, 1], F32)

        # ---- stage 0: weights transpose + hp + biases --------------------
        bh_sb = singles.tile([128, GC], F32)
        be_sb = singles.tile([128, GC], F32)
        nc.sync.dma_start(out=bh_sb[:], in_=b_h.rearrange("(c p) -> p c", p=128))
        nc.sync.dma_start(out=be_sb[:], in_=b_e.rearrange("(c p) -> p c", p=128))
        nc.sync.dma_start(out=v_sb[:], in_=v.rearrange("(c p) -> p c", p=128))
        nc.vector.tensor_add(bsum[:], bh_sb[:], be_sb[:])
        nc.gpsimd.memset(ones_bf[:], 1.0)
        vbc = singles.tile([128, GC, TOK], BF16)
        nc.gpsimd.memset(probs[:, 0:TOK], 1.0)
        for gc in range(GC):
            nc.vector.tensor_scalar_mul(
                vbc[:, gc, :], probs[:, 0:TOK], v_sb[:, gc : gc + 1]
            )

        with tc.tile_pool(name="stage0", bufs=4) as wload, tc.tile_pool(
            name="stage0_static", bufs=1
        ) as wstat, tc.tile_pool(name="trps", bufs=1, space="PSUM") as trps_pool:
            identity = wstat.tile([128, 128], F32, tag="ident")
            make_identity(nc, identity[:])
            WhT = wstat.tile([128, HC, H], F32, tag="wht")
            hidT = wstat.tile([128, HC, BC], F32, tag="hidt")

            for gc in range(GC):
                wn = wload.tile([128, H], F32, tag="wn")
                nc.sync.dma_start(out=wn[:], in_=w_e[ts(gc, 128), :])
                for hc in range(HC):
                    tp = trps_pool.tile([128, 128], F32, tag="tr")
                    nc.tensor.transpose(tp[:], wn[:, ts(hc, 128)], identity[:])
                    nc.vector.tensor_copy(WeT[:, hc, ts(gc, 128)], tp[:])
            for gc in range(GC):
                wn = wload.tile([128, H], F32, tag="wn")
                nc.sync.dma_start(out=wn[:], in_=w_h[ts(gc, 128), :])
                for hc in range(HC):
                    tp = trps_pool.tile([128, 128], F32, tag="tr")
                    nc.tensor.transpose(tp[:], wn[:, ts(hc, 128)], identity[:])
                    nc.vector.tensor_copy(WhT[:, hc, ts(gc, 128)], tp[:])

            hid_nat = wload.tile([BC, H], F32, tag="hid")
            nc.sync.dma_start(out=hid_nat[:], in_=hid[:, :])
            for hc in range(HC):
                tph = trps_pool.tile([128, BC], F32, tag="tr")
                nc.tensor.transpose(
                    tph[:], hid_nat[:, ts(hc, 128)], identity[0:BC, 0:BC]
                )
                nc.vector.tensor_copy(hidT[:, hc, :], tph[:])

            # hp^T[g, b] accumulated over h chunks (fp32, tiny N)
            for gc in range(GC):
                hp_ps = trps_pool.tile([128, BC], F32, tag="tr")
                for hc in range(HC):
                    nc.tensor.matmul(
                        hp_ps[:],
                        WhT[:, hc, ts(gc, 128)],
                        hidT[:, hc, :],
                        start=(hc == 0),
                        stop=(hc == HC - 1),
                    )
                nc.vector.tensor_scalar(
                    out=bias_all[:, gc, :],
                    in0=hp_ps[:],
                    scalar1=bsum[:, gc : gc + 1],
                    scalar2=None,
                    op0=ALU.add,
                )

        # ---- main loop: 16 groups of 512 tokens --------------------------
        # Pipelined: iteration g emits DMA loads for g+2, convert+XBAR
        # transpose for g+1, then the ep/tanh/z chain for g. The z-matmul
        # for g is deferred (carry) into iteration g+1 after two ep chunks
        # so the PE never waits on the trailing DVE z accumulation.
        n_total = reps * NGRP

        def issue_load(grp):
            g = grp % NGRP
            b = g // NGRP_PER_B
            t0 = (g % NGRP_PER_B) * TOK
            en = enc_pool.tile([128, SUB, H], F32, tag="en")
            nc.scalar.dma_start(
                out=en[:],
                in_=enc[t0 : t0 + TOK, b, :].rearrange("(s p) h -> p s h", p=128),
            )
            return en

        def issue_convert(en_nat):
            en_bf = encbf_pool.tile([128, SUB, H], BF16, tag="enbf")
            nc.scalar.copy(out=en_bf[:], in_=en_nat[:])
            return en_bf

        def issue_transposes(en_bf):
            encT = encT_pool.tile([128, HC, SUB, 128], BF16)
            for s in range(SUB):
                nc.sync.dma_start_transpose(out=encT[:, :, s, :], in_=en_bf[:, s, :])
            return encT

        loads = {0: issue_load(0), 1: issue_load(1), 2: issue_load(2)}
        bfs = {0: issue_convert(loads[0]), 1: issue_convert(loads[1])}
        encTs = {0: issue_transposes(bfs[0]), 1: issue_transposes(bfs[1])}
        carry = None  # deferred z reduction of the previous group

        def softmax_b(b):
            r = slice(32 * b, 32 * b + 1)
            # |scores| is bounded (sum of v*tanh terms), so exp without the
            # max subtraction is safe in f32 and matches softmax exactly.
            nc.scalar.activation(
                out=probs[r, :], in_=scores[r, :], func=AF.Exp,
                bias=0.0, scale=1.0, accum_out=sums[r],
            )
            nc.vector.reciprocal(out=rsum[r], in_=sums[r])
            nc.vector.tensor_scalar_mul(probs[r, :], probs[r, :], rsum[r])
            nc.scalar.dma_start(out=out[b : b + 1, :], in_=probs[r, :])

        def flush_carry(c):
            c_z, c_b, c_t0 = c
            sc_ps = sc_pool.tile([128, TOK], F32)
            r = slice(32 * c_b, 32 * c_b + 1)
            nc.tensor.matmul(
                sc_ps[r, :], ones_bf[:], c_z[:],
                start=True, stop=True, tile_position=(0, 32 * c_b),
            )
            nc.vector.tensor_copy(scores[r, c_t0 : c_t0 + TOK], sc_ps[r, :])

        for grp in range(n_total):
            g = grp % NGRP
            b = g // NGRP_PER_B
            t0 = (g % NGRP_PER_B) * TOK

            # prefetch: convert(g+2) first (single ACT instr), then the
            # load(g+3) dispatch; the XBARs for g+2 are emitted after the gc
            # loop so their sem waits can never block a sequencer mid-group.
            if grp + 2 < n_total:
                bfs[grp + 2] = issue_convert(loads.pop(grp + 2))
            if grp + 3 < n_total:
                loads[grp + 3] = issue_load(grp + 3)

            encT_cur = encTs.pop(grp)
            z2 = z_pool.tile([128, 2 * TOK], BF16)
            e_pair = None
            for gc in range(GC):
                ep_ps = ep_pool.tile([128, TOK], F32)
                for hc in range(HC):
                    nc.tensor.matmul(
                        ep_ps[:],
                        WeT[:, hc, ts(gc, 128)],
                        encT_cur[:, hc, :, :],
                        start=(hc == 0),
                        stop=(hc == HC - 1),
                    )
                if gc == 2 and carry is not None:
                    flush_carry(carry)
                    carry = None
                if gc % 2 == 0:
                    e_pair = energy_pool.tile([128, 2 * TOK], BF16)
                nc.scalar.activation(
                    out=e_pair[:, (gc % 2) * TOK : (gc % 2 + 1) * TOK],
                    in_=ep_ps[:],
                    func=AF.Tanh,
                    bias=bias_all[:, gc, b : b + 1],
                    scale=1.0,
                )
                if gc % 2 == 1:
                    k = gc // 2
                    if k == 0:
                        nc.vector.tensor_mul(
                            z2[:], e_pair[:], vbc[:, 0:2, :]
                        )
                    else:
                        zt = tmp_pool.tile([128, 2 * TOK], BF16)
                        nc.vector.tensor_mul(
                            zt[:], e_pair[:], vbc[:, 2 * k : 2 * k + 2, :]
                        )
                        nc.vector.tensor_add(z2[:], z2[:], zt[:])
            z = z_pool.tile([128, TOK], BF16, tag="zfold")
            nc.vector.tensor_add(z[:], z2[:, 0:TOK], z2[:, TOK:])
            carry = (z, b, t0)
            if grp + 2 < n_total:
                encTs[grp + 2] = issue_transposes(bfs.pop(grp + 2))

            if g == NGRP - 1:
                # end of a pass: flush the last group's scores, then run all
                # four softmaxes back-to-back (2 ACT table switches per pass
                # instead of 8, and no mid-group ACT serialization).
                flush_carry(carry)
                carry = None
                for bb in range(BC):
                    softmax_b(bb)


_NC_CACHE = None


def _get_nc():
    global _NC_CACHE
    if _NC_CACHE is None:
        _NC_CACHE = build_kernel_nc()
    return _NC_CACHE


def make_in_maps(hidden, encoder_outputs, W_h, b_h, W_e, b_e, v):
    hidden = np.asarray(hidden, dtype=np.float32)
    enc = np.asarray(encoder_outputs, dtype=np.float32)
    W_h = np.ascontiguousarray(np.asarray(W_h, dtype=np.float32))
    W_e = np.ascontiguousarray(np.asarray(W_e, dtype=np.float32))
    b_h = np.ascontiguousarray(np.asarray(b_h, dtype=np.float32))
    b_e = np.ascontiguousarray(np.asarray(b_e, dtype=np.float32))
    v = np.ascontiguousarray(np.asarray(v, dtype=np.float32))
    hid0 = hidden.reshape(B, H)
    in_maps = []
    for c in range(NCORES):
        in_maps.append(
            {
                "enc": np.ascontiguousarray(enc[:, c * BC : (c + 1) * BC, :]),
                "hid": np.ascontiguousarray(hid0[c * BC : (c + 1) * BC, :]),
                "W_e": W_e,
                "W_h": W_h,
                "b_h": b_h,
                "b_e": b_e,
                "v": v,
            }
        )
    return in_maps


def kernel(hidden, encoder_outputs, W_h, b_h, W_e, b_e, v):
    nc = _get_nc()
    in_maps = make_in_maps(hidden, encoder_outputs, W_h, b_h, W_e, b_e, v)
    res = run_bass_kernel_spmd(nc, in_maps, list(range(NCORES)))
    full = np.concatenate([res.results[c]["out"] for c in range(NCORES)], axis=0)
    return full[:, None, :].astype(np.float32)



# revision 2
# speedup vs baseline: 1.0215x; 1.0215x over previous
"""Bahdanau attention on 8 Trainium2 cores (Bass/Tile), data-parallel over B.

reference (per batch b, all shapes full):
    hp  = hidden[0] @ W_h.T + b_h                    # (B, H)
    ep  = einsum('tbh,gh->btg', enc, W_e) + b_e      # (B, T, H)
    en  = tanh(hp[:, None, :] + ep)                  # (B, T, H)
    sc  = en @ v                                     # (B, T)
    out = softmax(sc, -1)[:, None, :]                # (B, 1, T)

Sharding: B=32 split 4-per-core across 8 cores; weights replicated.

Design (HW-measured ~134us per main-loop pass vs ~283us for the
bf16/XBAR-transpose kernel this replaces):

  - Mixed-precision ep matmul keyed on |v|: the scores error contributed
    by output column g is weighted by |v_g| (scores = v . tanh(...)), so
    the host permutes the g axis (rows of W_e/W_h, entries of b_h/b_e/v)
    by DESCENDING |v_g|.  The top 384 columns (~85% of sum v^2) run in
    bf16; the low 640 columns run as fp8-e4m3 DoubleRowSwInterleave
    matmuls (2 weights per PE cell, 2x MACs/cycle; the host pre-reverses
    and A/B-interleaves each stationary block so the non-FWL weight load
    reads contiguously and hides under the 256-cycle matmul -- plain
    DoubleRow measured ~150us/pass, SwInterleave ~134us).  Measured rel
    err 1.40e-2 vs the 2e-2 gate (full-fp8 measures 3.4e-2 -- fails).
  - All weights are host-pretransposed/quantized/permuted into the exact
    SBUF [h_lo, hc, g] layouts (pure data marshalling, zero device work):
    W_e^T fp8 x256, W_e^T bf16, W_h^T bf16, b_h+b_e and v pre-chunked.
  - enc is host-pretransposed per (batch, 512-token group) into the
    moving-tensor layout [h_lo(128), hc, t], so the kernel needs NO
    on-device transpose at all: the old f32->bf16-convert -> XBAR
    -> fp8-cast chain serialized on the DMA fabric and gated every
    group (the cost-model trace showed the pipeline locked to a
    15.5us/group DMA+dependency cycle; eliminating the XBAR collapsed
    it to ~5.2us/group).
  - Per 512-token group: one 2MB DMA (SP queue) loads enc f32; ACT
    converts f32->bf16 with the x16 enc scale folded in; DVE casts
    bf16->fp8 (x256 W scale pre-applied on host; tanh divides out
    16*256=4096 via its scale operand).  PE runs 3 bf16 chunks (8
    matmuls each), the deferred ones-matmul score reduction for the
    previous group (tile_position lands batch b's scores on partition
    32b), then 5 fp8-DoubleRow chunks (4 matmuls each, K=256).
  - tanh on ACT (PSUM f32 -> bf16) with per-partition bias hp^T+b_h+b_e;
    the v-weighted reduction over g runs as 8 chained per-chunk FMAs on
    DVE (scalar_tensor_tensor, f32 accumulator, one final bf16 round).
  - Prefetch tail per iteration: ACT convert for g+2, DVE cast for g+1,
    enc DMA for g+3 at the top -- each a full iteration ahead of its
    consumer so the (scheduler-reordered) in-order engine streams never
    head-of-line block the compute chain.
  - Engine placement keeps every queue under the PE time: enc DMA on the
    otherwise-idle SP queue (NOT the ACT queue -- a DMA occupies the
    issuing queue for the whole transfer), out DMA on Pool, scores copy
    on ACT.  GPSIMD cannot touch PSUM and is ~10x software-slow for
    dtype-converting tensor ops, so it only does memset + out-DMAs.
    fp32r moving operands measured 2x slower than bf16 on HW (rejected).
  - softmax skips the max-subtraction (scores are bounded v.tanh sums,
    so f32 exp is exact-safe) and runs once per pass for all batches.
"""

import sys
from contextlib import ExitStack

import numpy as np
import ml_dtypes

try:
    import concourse  # noqa: F401
except ImportError:  # pragma: no cover
    sys.path.insert(0, "/opt/trn_rl_repo")

import concourse.tile as tile
from concourse import bacc, mybir
from concourse.bass import ts
from concourse.bass_utils import run_bass_kernel_spmd

H = 1024
T = 2048
B = 32
NCORES = 8
BC = B // NCORES          # batches per core
HC = H // 128             # h chunks (contraction)
GC = H // 128             # g chunks (output, |v|-sorted descending)
KF8 = 5                   # low-|v| g chunks computed in fp8 DoubleRow
KBF = GC - KF8            # bf16 chunks (top |v|), run FIRST in each group
G8 = KF8 * 128            # fp8 columns
GBF = H - G8              # bf16 columns
TOK = 512                 # tokens per group (one batch each)
NGRP_PER_B = T // TOK
NGRP = BC * NGRP_PER_B
ESCL = 16.0               # enc scale into bf16/fp8 (keeps e4m3 out of denormals)
WSCL = 256.0              # W_e fp8 scale (W ~ uniform(-1/32,1/32))

F32 = mybir.dt.float32
BF16 = mybir.dt.bfloat16
F8E4 = mybir.dt.float8e4
AF = mybir.ActivationFunctionType
ALU = mybir.AluOpType
PM = mybir.MatmulPerfMode


def build_kernel_nc(reps=1):
    nc = bacc.Bacc(
        "TRN2",
        target_bir_lowering=False,
        debug=False,
        enable_asserts=False,
        num_devices=NCORES,
    )
    enc = nc.dram_tensor("encP", [NGRP, 128, HC * TOK], F32, kind="ExternalInput").ap()
    hidT = nc.dram_tensor("hidT", [128, HC * BC], BF16, kind="ExternalInput").ap()
    whT = nc.dram_tensor("WhT", [128, HC * H], BF16, kind="ExternalInput").ap()
    weT8 = nc.dram_tensor("WeT8", [128, HC * G8], F8E4, kind="ExternalInput").ap()
    weTb = nc.dram_tensor("WeTb", [128, HC * GBF], BF16, kind="ExternalInput").ap()
    bsum = nc.dram_tensor("bsum", [128, GC], F32, kind="ExternalInput").ap()
    vpg = nc.dram_tensor("vpg", [128, GC], F32, kind="ExternalInput").ap()
    out = nc.dram_tensor("out", [BC, T], F32, kind="ExternalOutput").ap()

    with tile.TileContext(nc) as tc:
        _kernel_body(tc, enc, hidT, whT, weT8, weTb, bsum, vpg, out, reps=reps)
    nc.compile()
    return nc


def _kernel_body(tc, enc, hidT_d, whT_d, weT8_d, weTb_d, bsum_d, vpg_d, out, reps=1):
    nc = tc.nc
    with ExitStack() as ctx:
        singles = ctx.enter_context(tc.tile_pool(name="singles", bufs=1))
        enc_pool = ctx.enter_context(tc.tile_pool(name="enc_f32", bufs=3))
        encT_pool = ctx.enter_context(tc.tile_pool(name="encT", bufs=3))
        encT8_pool = ctx.enter_context(tc.tile_pool(name="encT8", bufs=3))
        energy_pool = ctx.enter_context(tc.tile_pool(name="energy", bufs=6))
        zf_pool = ctx.enter_context(tc.tile_pool(name="zf32", bufs=2))
        zb_pool = ctx.enter_context(tc.tile_pool(name="zb16", bufs=3))
        ep_pool = ctx.enter_context(tc.tile_pool(name="epps", bufs=5, space="PSUM"))
        sc_pool = ctx.enter_context(tc.tile_pool(name="scps", bufs=2, space="PSUM"))

        # ---- persistent SBUF tensors -------------------------------------
        WeT8 = singles.tile([128, HC // 2, KF8, 2, 128], F8E4)  # SwInterleave blocks
        WeTb = singles.tile([128, HC, GBF], BF16)  # [h_lo, hc, g']
        bias_all = singles.tile([128, GC, BC], F32)  # hp^T + b_h + b_e (per g')
        v_sb = singles.tile([128, GC], F32)
        ones_bf = singles.tile([128, 1], BF16)
        bsum_sb = singles.tile([128, GC], F32)
        scores = singles.tile([128, T], F32)       # batch b at partition 32*b
        probs = singles.tile([128, T], F32)
        sums = singles.tile([128, 1], F32)
        rsum = singles.tile([128, 1], F32)

        # ---- stage 0: weight DMAs (host-pretransposed) + hp --------------
        nc.sync.dma_start(out=WeT8[:], in_=weT8_d.rearrange("p (j g i m) -> p j g i m", j=HC // 2, g=KF8, i=2))
        nc.sync.dma_start(out=WeTb[:], in_=weTb_d.rearrange("p (c g) -> p c g", c=HC))
        nc.sync.dma_start(out=bsum_sb[:], in_=bsum_d[:, :])
        nc.sync.dma_start(out=v_sb[:], in_=vpg_d[:, :])
        nc.gpsimd.memset(ones_bf[:], 1.0)

        with tc.tile_pool(name="stage0", bufs=1) as wstat, tc.tile_pool(
            name="hpps", bufs=1, space="PSUM"
        ) as hp_pool:
            WhT = wstat.tile([128, HC, H], BF16, tag="wht")
            hidT = wstat.tile([128, HC, BC], BF16, tag="hidt")
            nc.sync.dma_start(out=WhT[:], in_=whT_d.rearrange("p (c g) -> p c g", c=HC))
            nc.sync.dma_start(out=hidT[:], in_=hidT_d.rearrange("p (c b) -> p c b", c=HC))
            for gc in range(GC):
                hp_ps = hp_pool.tile([128, BC], F32, tag="hp")
                for hc in range(HC):
                    nc.tensor.matmul(
                        hp_ps[:],
                        WhT[:, hc, ts(gc, 128)],
                        hidT[:, hc, :],
                        start=(hc == 0),
                        stop=(hc == HC - 1),
                    )
                nc.vector.tensor_scalar(
                    out=bias_all[:, gc, :],
                    in0=hp_ps[:],
                    scalar1=bsum_sb[:, gc : gc + 1],
                    scalar2=None,
                    op0=ALU.add,
                )

        # ---- main loop: 16 groups of 512 tokens --------------------------
        n_total = reps * NGRP

        def issue_load(grp):
            g = grp % NGRP
            en = enc_pool.tile([128, HC, TOK], F32, tag="en")
            nc.sync.dma_start(
                out=en[:], in_=enc[g, :, :].rearrange("p (c t) -> p c t", c=HC)
            )
            return en

        def issue_convert(en_f32):
            # f32 -> bf16 with the x16 enc scale folded in (ACT); the host
            # already staged enc in the transposed [h_lo, hc, t] layout.
            encT = encT_pool.tile([128, HC, TOK], BF16)
            nc.scalar.activation(
                out=encT[:], in_=en_f32[:], func=AF.Copy, bias=0.0, scale=ESCL
            )
            return encT

        def issue_f8cast(encT):
            # pure dtype cast bf16 -> fp8 (enc already x16-scaled) on DVE
            encT8 = encT8_pool.tile([128, HC, TOK], F8E4)
            nc.vector.tensor_copy(encT8[:], encT[:])
            return encT8

        loads = {g: issue_load(g) for g in range(3)}
        encTs = {0: issue_convert(loads[0]), 1: issue_convert(loads[1])}
        encT8s = {0: issue_f8cast(encTs[0])}
        carry = None  # deferred z reduction of the previous group

        def softmax_b(b):
            r = slice(32 * b, 32 * b + 1)
            # |scores| is bounded (sum of v*tanh terms), so exp without the
            # max subtraction is safe in f32 and matches softmax exactly.
            nc.scalar.activation(
                out=probs[r, :], in_=scores[r, :], func=AF.Exp,
                bias=0.0, scale=1.0, accum_out=sums[r],
            )
            nc.vector.reciprocal(out=rsum[r], in_=sums[r])
            nc.vector.tensor_scalar_mul(probs[r, :], probs[r, :], rsum[r])
            nc.gpsimd.dma_start(out=out[b : b + 1, :], in_=probs[r, :])

        def flush_carry(c):
            c_z, c_b, c_t0 = c
            sc_ps = sc_pool.tile([128, TOK], F32)
            r = slice(32 * c_b, 32 * c_b + 1)
            nc.tensor.matmul(
                sc_ps[r, :], ones_bf[:], c_z[:],
                start=True, stop=True, tile_position=(0, 32 * c_b),
            )
            nc.scalar.copy(out=scores[r, c_t0 : c_t0 + TOK], in_=sc_ps[r, :])

        for grp in range(n_total):
            g = grp % NGRP
            b = g // NGRP_PER_B
            t0 = (g % NGRP_PER_B) * TOK

            if grp + 3 < n_total:
                loads[grp + 3] = issue_load(grp + 3)

            encT_cur = encTs.pop(grp)
            encT8_cur = encT8s.pop(grp)
            for gc in range(GC):
                ep_ps = ep_pool.tile([128, TOK], F32)
                if gc >= KBF:
                    for j in range(HC // 2):
                        nc.tensor.matmul(
                            ep_ps[:],
                            WeT8[:, j, gc - KBF, :, :],
                            encT8_cur[:, 2 * j : 2 * j + 2, :],
                            start=(j == 0),
                            stop=(j == HC // 2 - 1),
                            perf_mode=PM.DoubleRowSwInterleave,
                        )
                    tanh_scale = 1.0 / (ESCL * WSCL)
                else:
                    for hc in range(HC):
                        nc.tensor.matmul(
                            ep_ps[:],
                            WeTb[:, hc, ts(gc, 128)],
                            encT_cur[:, hc, :],
                            start=(hc == 0),
                            stop=(hc == HC - 1),
                        )
                    tanh_scale = 1.0 / ESCL
                if gc == KBF and carry is not None:
                    # flush at the bf16->fp8 boundary (no extra mode switch)
                    flush_carry(carry)
                    carry = None
                e_ch = energy_pool.tile([128, TOK], BF16)
                nc.scalar.activation(
                    out=e_ch[:],
                    in_=ep_ps[:],
                    func=AF.Tanh,
                    bias=bias_all[:, gc, b : b + 1],
                    scale=tanh_scale,
                )
                # z += v_chunk * energy_chunk, f32 accumulation on DVE; the
                # last chunk rounds once to bf16 for the ones-matmul.
                if gc == 0:
                    zf = zf_pool.tile([128, TOK], F32)
                    nc.vector.tensor_scalar_mul(zf[:], e_ch[:], v_sb[:, 0:1])
                elif gc < GC - 1:
                    zf_new = zf_pool.tile([128, TOK], F32)
                    nc.vector.scalar_tensor_tensor(
                        out=zf_new[:], in0=e_ch[:],
                        scalar=v_sb[:, gc : gc + 1], in1=zf[:],
                        op0=ALU.mult, op1=ALU.add,
                    )
                    zf = zf_new
                else:
                    zb = zb_pool.tile([128, TOK], BF16)
                    nc.vector.scalar_tensor_tensor(
                        out=zb[:], in0=e_ch[:],
                        scalar=v_sb[:, gc : gc + 1], in1=zf[:],
                        op0=ALU.mult, op1=ALU.add,
                    )
            carry = (zb, b, t0)
            # prefetch tail: ACT convert for g+2, DVE fp8 cast for g+1 —
            # each a full iteration ahead of its consumer, so even
            # scheduler-reordered they never block the compute chain.
            if grp + 2 < n_total:
                encTs[grp + 2] = issue_convert(loads.pop(grp + 2))
            if grp + 1 < n_total:
                encT8s[grp + 1] = issue_f8cast(encTs[grp + 1])

            if g == NGRP - 1:
                flush_carry(carry)
                carry = None
                for bb in range(BC):
                    softmax_b(bb)


_NC_CACHE = None


def _get_nc():
    global _NC_CACHE
    if _NC_CACHE is None:
        _NC_CACHE = build_kernel_nc()
    return _NC_CACHE


E4M3 = ml_dtypes.float8_e4m3


def make_in_maps(hidden, encoder_outputs, W_h, b_h, W_e, b_e, v):
    """Host-side marshalling: shard over B, |v|-sort the g axis, transpose
    weights and enc into the SBUF layouts, quantize the fp8 weight part."""
    hidden = np.asarray(hidden, dtype=np.float32)
    enc = np.asarray(encoder_outputs, dtype=np.float32)
    W_h = np.asarray(W_h, dtype=np.float32)
    W_e = np.asarray(W_e, dtype=np.float32)
    b_h = np.asarray(b_h, dtype=np.float32)
    b_e = np.asarray(b_e, dtype=np.float32)
    v = np.asarray(v, dtype=np.float32)

    perm = np.argsort(np.abs(v), kind="stable")[::-1]   # descending |v|
    Wep = W_e[perm]
    Whp = W_h[perm]
    bsum_p = (b_h + b_e)[perm]
    vp = v[perm]

    def to_sb(mat_T, dtype):             # mat_T: (H_in, G) = W^T
        a = mat_T.reshape(HC, 128, -1).transpose(1, 0, 2)  # (128, HC, G)
        return np.ascontiguousarray(
            a.reshape(128, -1).astype(dtype, copy=False)
        )

    WeT = Wep.T                          # (h, g'), |v| descending over g'
    weTb = to_sb(np.asarray(WeT[:, :GBF], dtype=ml_dtypes.bfloat16),
                 ml_dtypes.bfloat16)
    # fp8 stationary blocks pre-interleaved for DoubleRowSwInterleave:
    # per (j, g-chunk) block, flat[p, 2*(127-m)+i] = W'[g0+m, (2j+i)*128+p]
    w8 = np.asarray(WeT[:, GBF:] * WSCL, dtype=E4M3)     # (H, G8)
    w8 = w8.reshape(HC // 2, 2, 128, KF8, 128)           # [j, i, p, gc, m]
    ilv = np.empty((128, HC // 2, KF8, 256), dtype=E4M3) # [p, j, gc, k]
    for m in range(128):
        for i in range(2):
            ilv[:, :, :, 2 * (127 - m) + i] = w8[:, i, :, :, m].transpose(1, 0, 2)
    weT8 = np.ascontiguousarray(ilv.reshape(128, -1))
    whT = to_sb(np.asarray(Whp.T, dtype=ml_dtypes.bfloat16), ml_dtypes.bfloat16)
    bsum_sb = np.ascontiguousarray(bsum_p.reshape(GC, 128).T)
    vpg = np.ascontiguousarray(vp.reshape(GC, 128).T)

    def encP_core(c):
        x = enc[:, c * BC : (c + 1) * BC, :]            # (T, BC, H)
        x = x.reshape(NGRP_PER_B, TOK, BC, HC, 128)     # [tg, t, b, hc, p]
        x = x.transpose(2, 0, 4, 3, 1)                  # [b, tg, p, hc, t]
        return np.ascontiguousarray(x.reshape(NGRP, 128, HC * TOK))

    hid0 = hidden.reshape(B, H)
    in_maps = []
    for c in range(NCORES):
        hc_slice = hid0[c * BC : (c + 1) * BC, :]       # (BC, H)
        hidT = np.ascontiguousarray(
            hc_slice.T.reshape(HC, 128, BC).transpose(1, 0, 2)
            .reshape(128, -1).astype(ml_dtypes.bfloat16)
        )
        in_maps.append(
            {
                "encP": encP_core(c),
                "hidT": hidT,
                "WhT": whT,
                "WeT8": weT8,
                "WeTb": weTb,
                "bsum": bsum_sb,
                "vpg": vpg,
            }
        )
    return in_maps


def kernel(hidden, encoder_outputs, W_h, b_h, W_e, b_e, v):
    nc = _get_nc()
    in_maps = make_in_maps(hidden, encoder_outputs, W_h, b_h, W_e, b_e, v)
    res = run_bass_kernel_spmd(nc, in_maps, list(range(NCORES)))
    full = np.concatenate([res.results[c]["out"] for c in range(NCORES)], axis=0)
    return full[:, None, :].astype(np.float32)


# revision 3
# speedup vs baseline: 1.2739x; 1.2471x over previous
"""Bahdanau attention on 8 Trainium2 cores (Bass/Tile), data-parallel over B.

reference (per batch b, all shapes full):
    hp  = hidden[0] @ W_h.T + b_h                    # (B, H)
    ep  = einsum('tbh,gh->btg', enc, W_e) + b_e      # (B, T, H)
    en  = tanh(hp[:, None, :] + ep)                  # (B, T, H)
    sc  = en @ v                                     # (B, T)
    out = softmax(sc, -1)[:, None, :]                # (B, 1, T)

Sharding: B=32 split 4-per-core across 8 cores; weights replicated.

Design (HW-measured ~82us per main-loop pass vs ~283us for the previous
bf16/XBAR-transpose kernel):

  - Mixed-precision ep matmul keyed on |v|: the scores error contributed
    by output column g is weighted by |v_g| (scores = v . tanh(...)), so
    the host permutes the g axis (rows of W_e/W_h, entries of b_h/b_e/v)
    by DESCENDING |v_g|.  The top 384 columns (~85% of sum v^2) run in
    bf16; the low 640 columns run as fp8-e4m3 DoubleRow matmuls (2
    weights per PE cell, 2x MACs/cycle).  Measured rel err 1.40e-2 vs
    the 2e-2 gate (full-fp8 measures 3.4e-2 -- fails).
  - All weights are host-pretransposed/quantized/permuted into the exact
    SBUF [h_lo, hc, g] layouts (pure data marshalling, zero device work):
    W_e^T fp8 x256, W_e^T bf16, W_h^T bf16, b_h+b_e and v pre-chunked.
  - enc is host-pretransposed per (batch, 512-token group) into the
    moving-tensor layout [h_lo(128), hc, t], so the kernel needs NO
    on-device transpose at all: the old f32->bf16-convert -> XBAR
    -> fp8-cast chain serialized on the DMA fabric and gated every
    group (the cost-model trace showed the pipeline locked to a
    15.5us/group DMA+dependency cycle; eliminating the XBAR collapsed
    it to ~5.2us/group).
  - Per 512-token group: one 2MB DMA (SP queue) loads enc f32; ACT
    converts f32->bf16 with the x16 enc scale folded in; DVE casts
    bf16->fp8 (x256 W scale pre-applied on host; tanh divides out
    16*256=4096 via its scale operand).  PE runs 3 bf16 chunks (8
    matmuls each), the deferred ones-matmul score reduction for the
    previous group (tile_position lands batch b's scores on partition
    32b), then 5 fp8-DoubleRow chunks (4 matmuls each, K=256).
  - tanh on ACT (PSUM f32 -> bf16) with per-partition bias hp^T+b_h+b_e;
    the v-weighted reduction over g runs as 8 chained per-chunk FMAs on
    DVE (scalar_tensor_tensor, f32 accumulator, one final bf16 round).
  - Prefetch tail per iteration: ACT convert for g+2, DVE cast for g+1,
    enc DMA for g+3 at the top -- each a full iteration ahead of its
    consumer so the (scheduler-reordered) in-order engine streams never
    head-of-line block the compute chain.
  - Engine placement keeps every queue under the PE time: enc DMA on the
    otherwise-idle SP queue (NOT the ACT queue -- a DMA occupies the
    issuing queue for the whole transfer), out DMA on Pool, scores copy
    on DVE, and the f32->bf16 convert split 6/2 between ACT and DVE to
    balance the two near-critical engines.  GPSIMD cannot touch PSUM and
    is ~10x software-slow for dtype-converting tensor ops, so it only
    does memset + out-DMAs.  fp32r moving operands measured 2x slower
    than bf16 on HW (rejected).
  - softmax skips the max-subtraction (scores are bounded v.tanh sums,
    so f32 exp is exact-safe) and runs once per pass for all batches.
"""

import sys
from contextlib import ExitStack

import numpy as np
import ml_dtypes

try:
    import concourse  # noqa: F401
except ImportError:  # pragma: no cover
    sys.path.insert(0, "/opt/trn_rl_repo")

import concourse.tile as tile
from concourse import bacc, mybir
from concourse.bass import ts
from concourse.bass_utils import run_bass_kernel_spmd

H = 1024
T = 2048
B = 32
NCORES = 8
BC = B // NCORES          # batches per core
HC = H // 128             # h chunks (contraction)
GC = H // 128             # g chunks (output, |v|-sorted descending)
KF8 = 5                   # low-|v| g chunks computed in fp8 DoubleRow
KBF = GC - KF8            # bf16 chunks (top |v|), run FIRST in each group
G8 = KF8 * 128            # fp8 columns
GBF = H - G8              # bf16 columns
TOK = 512                 # tokens per group (one batch each)
NGRP_PER_B = T // TOK
NGRP = BC * NGRP_PER_B
ESCL = 16.0               # enc scale into bf16/fp8 (keeps e4m3 out of denormals)
WSCL = 256.0              # W_e fp8 scale (W ~ uniform(-1/32,1/32))

F32 = mybir.dt.float32
BF16 = mybir.dt.bfloat16
F8E4 = mybir.dt.float8e4
AF = mybir.ActivationFunctionType
ALU = mybir.AluOpType
PM = mybir.MatmulPerfMode


def build_kernel_nc(reps=1):
    nc = bacc.Bacc(
        "TRN2",
        target_bir_lowering=False,
        debug=False,
        enable_asserts=False,
        num_devices=NCORES,
    )
    enc = nc.dram_tensor("encP", [NGRP, 128, HC * TOK], F32, kind="ExternalInput").ap()
    hidT = nc.dram_tensor("hidT", [128, HC * BC], BF16, kind="ExternalInput").ap()
    whT = nc.dram_tensor("WhT", [128, HC * H], BF16, kind="ExternalInput").ap()
    weT8 = nc.dram_tensor("WeT8", [128, HC * G8], F8E4, kind="ExternalInput").ap()
    weTb = nc.dram_tensor("WeTb", [128, HC * GBF], BF16, kind="ExternalInput").ap()
    bsum = nc.dram_tensor("bsum", [128, GC], F32, kind="ExternalInput").ap()
    vpg = nc.dram_tensor("vpg", [128, GC], F32, kind="ExternalInput").ap()
    out = nc.dram_tensor("out", [BC, T], F32, kind="ExternalOutput").ap()

    with tile.TileContext(nc) as tc:
        _kernel_body(tc, enc, hidT, whT, weT8, weTb, bsum, vpg, out, reps=reps)
    nc.compile()
    return nc


def _kernel_body(tc, enc, hidT_d, whT_d, weT8_d, weTb_d, bsum_d, vpg_d, out, reps=1):
    nc = tc.nc
    with ExitStack() as ctx:
        singles = ctx.enter_context(tc.tile_pool(name="singles", bufs=1))
        enc_pool = ctx.enter_context(tc.tile_pool(name="enc_f32", bufs=3))
        encT_pool = ctx.enter_context(tc.tile_pool(name="encT", bufs=3))
        encT8_pool = ctx.enter_context(tc.tile_pool(name="encT8", bufs=3))
        energy_pool = ctx.enter_context(tc.tile_pool(name="energy", bufs=6))
        zf_pool = ctx.enter_context(tc.tile_pool(name="zf32", bufs=2))
        zb_pool = ctx.enter_context(tc.tile_pool(name="zb16", bufs=3))
        ep_pool = ctx.enter_context(tc.tile_pool(name="epps", bufs=5, space="PSUM"))
        sc_pool = ctx.enter_context(tc.tile_pool(name="scps", bufs=2, space="PSUM"))

        # ---- persistent SBUF tensors -------------------------------------
        WeT8 = singles.tile([128, HC // 2, KF8, 2, 128], F8E4)  # SwInterleave blocks
        WeTb = singles.tile([128, HC, GBF], BF16)  # [h_lo, hc, g']
        bias_all = singles.tile([128, GC, BC], F32)  # hp^T + b_h + b_e (per g')
        v_sb = singles.tile([128, GC], F32)
        ones_bf = singles.tile([128, 1], BF16)
        bsum_sb = singles.tile([128, GC], F32)
        scores = singles.tile([128, T], F32)       # batch b at partition 32*b
        probs = singles.tile([128, T], F32)
        sums = singles.tile([128, 1], F32)
        rsum = singles.tile([128, 1], F32)

        # ---- stage 0: weight DMAs (host-pretransposed) + hp --------------
        nc.sync.dma_start(out=WeT8[:], in_=weT8_d.rearrange("p (j g i m) -> p j g i m", j=HC // 2, g=KF8, i=2))
        nc.sync.dma_start(out=WeTb[:], in_=weTb_d.rearrange("p (c g) -> p c g", c=HC))
        nc.sync.dma_start(out=bsum_sb[:], in_=bsum_d[:, :])
        nc.sync.dma_start(out=v_sb[:], in_=vpg_d[:, :])
        nc.gpsimd.memset(ones_bf[:], 1.0)

        with tc.tile_pool(name="stage0", bufs=1) as wstat, tc.tile_pool(
            name="hpps", bufs=1, space="PSUM"
        ) as hp_pool:
            WhT = wstat.tile([128, HC, H], BF16, tag="wht")
            hidT = wstat.tile([128, HC, BC], BF16, tag="hidt")
            nc.sync.dma_start(out=WhT[:], in_=whT_d.rearrange("p (c g) -> p c g", c=HC))
            nc.sync.dma_start(out=hidT[:], in_=hidT_d.rearrange("p (c b) -> p c b", c=HC))
            for gc in range(GC):
                hp_ps = hp_pool.tile([128, BC], F32, tag="hp")
                for hc in range(HC):
                    nc.tensor.matmul(
                        hp_ps[:],
                        WhT[:, hc, ts(gc, 128)],
                        hidT[:, hc, :],
                        start=(hc == 0),
                        stop=(hc == HC - 1),
                    )
                nc.vector.tensor_scalar(
                    out=bias_all[:, gc, :],
                    in0=hp_ps[:],
                    scalar1=bsum_sb[:, gc : gc + 1],
                    scalar2=None,
                    op0=ALU.add,
                )

        # ---- main loop: 16 groups of 512 tokens --------------------------
        n_total = reps * NGRP

        def issue_load(grp):
            g = grp % NGRP
            en = enc_pool.tile([128, HC, TOK], F32, tag="en")
            nc.sync.dma_start(
                out=en[:], in_=enc[g, :, :].rearrange("p (c t) -> p c t", c=HC)
            )
            return en

        def issue_convert(en_f32):
            # f32 -> bf16 with the x16 enc scale folded in; split 6/2 over
            # ACT and DVE to balance the two near-critical engines.  The
            # host already staged enc in the transposed [h_lo, hc, t] layout.
            encT = encT_pool.tile([128, HC, TOK], BF16)
            nc.scalar.activation(
                out=encT[:, 0:6, :], in_=en_f32[:, 0:6, :],
                func=AF.Copy, bias=0.0, scale=ESCL,
            )
            nc.vector.tensor_scalar_mul(
                encT[:, 6:HC, :], en_f32[:, 6:HC, :], ESCL
            )
            return encT

        def issue_f8cast(encT):
            # pure dtype cast bf16 -> fp8 (enc already x16-scaled) on DVE
            encT8 = encT8_pool.tile([128, HC, TOK], F8E4)
            nc.vector.tensor_copy(encT8[:], encT[:])
            return encT8

        loads = {g: issue_load(g) for g in range(3)}
        encTs = {0: issue_convert(loads[0]), 1: issue_convert(loads[1])}
        encT8s = {0: issue_f8cast(encTs[0])}
        carry = None  # deferred z reduction of the previous group

        def softmax_b(b):
            r = slice(32 * b, 32 * b + 1)
            # |scores| is bounded (sum of v*tanh terms), so exp without the
            # max subtraction is safe in f32 and matches softmax exactly.
            nc.scalar.activation(
                out=probs[r, :], in_=scores[r, :], func=AF.Exp,
                bias=0.0, scale=1.0, accum_out=sums[r],
            )
            nc.vector.reciprocal(out=rsum[r], in_=sums[r])
            nc.vector.tensor_scalar_mul(probs[r, :], probs[r, :], rsum[r])
            nc.gpsimd.dma_start(out=out[b : b + 1, :], in_=probs[r, :])

        def flush_carry(c):
            c_z, c_b, c_t0 = c
            sc_ps = sc_pool.tile([128, TOK], F32)
            r = slice(32 * c_b, 32 * c_b + 1)
            nc.tensor.matmul(
                sc_ps[r, :], ones_bf[:], c_z[:],
                start=True, stop=True, tile_position=(0, 32 * c_b),
            )
            nc.vector.tensor_copy(scores[r, c_t0 : c_t0 + TOK], sc_ps[r, :])

        for grp in range(n_total):
            g = grp % NGRP
            b = g // NGRP_PER_B
            t0 = (g % NGRP_PER_B) * TOK

            if grp + 3 < n_total:
                loads[grp + 3] = issue_load(grp + 3)

            encT_cur = encTs.pop(grp)
            encT8_cur = encT8s.pop(grp)
            for gc in range(GC):
                ep_ps = ep_pool.tile([128, TOK], F32)
                if gc >= KBF:
                    for j in range(HC // 2):
                        nc.tensor.matmul(
                            ep_ps[:],
                            WeT8[:, j, gc - KBF, :, :],
                            encT8_cur[:, 2 * j : 2 * j + 2, :],
                            start=(j == 0),
                            stop=(j == HC // 2 - 1),
                            perf_mode=PM.DoubleRowSwInterleave,
                        )
                    tanh_scale = 1.0 / (ESCL * WSCL)
                else:
                    for hc in range(HC):
                        nc.tensor.matmul(
                            ep_ps[:],
                            WeTb[:, hc, ts(gc, 128)],
                            encT_cur[:, hc, :],
                            start=(hc == 0),
                            stop=(hc == HC - 1),
                        )
                    tanh_scale = 1.0 / ESCL
                if gc == KBF and carry is not None:
                    # flush at the bf16->fp8 boundary (no extra mode switch)
                    flush_carry(carry)
                    carry = None
                e_ch = energy_pool.tile([128, TOK], BF16)
                nc.scalar.activation(
                    out=e_ch[:],
                    in_=ep_ps[:],
                    func=AF.Tanh,
                    bias=bias_all[:, gc, b : b + 1],
                    scale=tanh_scale,
                )
                # z += v_chunk * energy_chunk, f32 accumulation on DVE; the
                # last chunk rounds once to bf16 for the ones-matmul.
                if gc == 0:
                    zf = zf_pool.tile([128, TOK], F32)
                    nc.vector.tensor_scalar_mul(zf[:], e_ch[:], v_sb[:, 0:1])
                elif gc < GC - 1:
                    zf_new = zf_pool.tile([128, TOK], F32)
                    nc.vector.scalar_tensor_tensor(
                        out=zf_new[:], in0=e_ch[:],
                        scalar=v_sb[:, gc : gc + 1], in1=zf[:],
                        op0=ALU.mult, op1=ALU.add,
                    )
                    zf = zf_new
                else:
                    zb = zb_pool.tile([128, TOK], BF16)
                    nc.vector.scalar_tensor_tensor(
                        out=zb[:], in0=e_ch[:],
                        scalar=v_sb[:, gc : gc + 1], in1=zf[:],
                        op0=ALU.mult, op1=ALU.add,
                    )
            carry = (zb, b, t0)
            # prefetch tail: ACT convert for g+2, DVE fp8 cast for g+1 —
            # each a full iteration ahead of its consumer, so even
            # scheduler-reordered they never block the compute chain.
            if grp + 2 < n_total:
                encTs[grp + 2] = issue_convert(loads.pop(grp + 2))
            if grp + 1 < n_total:
                encT8s[grp + 1] = issue_f8cast(encTs[grp + 1])

            if g == NGRP - 1:
                flush_carry(carry)
                carry = None
                for bb in range(BC):
                    softmax_b(bb)


_NC_CACHE = None


def _get_nc():
    global _NC_CACHE
    if _NC_CACHE is None:
        _NC_CACHE = build_kernel_nc()
    return _NC_CACHE


E4M3 = ml_dtypes.float8_e4m3


def make_in_maps(hidden, encoder_outputs, W_h, b_h, W_e, b_e, v):
    """Host-side marshalling: shard over B, |v|-sort the g axis, transpose
    weights and enc into the SBUF layouts, quantize the fp8 weight part."""
    hidden = np.asarray(hidden, dtype=np.float32)
    enc = np.asarray(encoder_outputs, dtype=np.float32)
    W_h = np.asarray(W_h, dtype=np.float32)
    W_e = np.asarray(W_e, dtype=np.float32)
    b_h = np.asarray(b_h, dtype=np.float32)
    b_e = np.asarray(b_e, dtype=np.float32)
    v = np.asarray(v, dtype=np.float32)

    perm = np.argsort(np.abs(v), kind="stable")[::-1]   # descending |v|
    Wep = W_e[perm]
    Whp = W_h[perm]
    bsum_p = (b_h + b_e)[perm]
    vp = v[perm]

    def to_sb(mat_T, dtype):             # mat_T: (H_in, G) = W^T
        a = mat_T.reshape(HC, 128, -1).transpose(1, 0, 2)  # (128, HC, G)
        return np.ascontiguousarray(
            a.reshape(128, -1).astype(dtype, copy=False)
        )

    WeT = Wep.T                          # (h, g'), |v| descending over g'
    weTb = to_sb(np.asarray(WeT[:, :GBF], dtype=ml_dtypes.bfloat16),
                 ml_dtypes.bfloat16)
    # fp8 stationary blocks pre-interleaved for DoubleRowSwInterleave:
    # per (j, g-chunk) block, flat[p, 2*(127-m)+i] = W'[g0+m, (2j+i)*128+p]
    w8 = np.asarray(WeT[:, GBF:] * WSCL, dtype=E4M3)     # (H, G8)
    w8 = w8.reshape(HC // 2, 2, 128, KF8, 128)           # [j, i, p, gc, m]
    ilv = np.empty((128, HC // 2, KF8, 256), dtype=E4M3) # [p, j, gc, k]
    for m in range(128):
        for i in range(2):
            ilv[:, :, :, 2 * (127 - m) + i] = w8[:, i, :, :, m].transpose(1, 0, 2)
    weT8 = np.ascontiguousarray(ilv.reshape(128, -1))
    whT = to_sb(np.asarray(Whp.T, dtype=ml_dtypes.bfloat16), ml_dtypes.bfloat16)
    bsum_sb = np.ascontiguousarray(bsum_p.reshape(GC, 128).T)
    vpg = np.ascontiguousarray(vp.reshape(GC, 128).T)

    def encP_core(c):
        x = enc[:, c * BC : (c + 1) * BC, :]            # (T, BC, H)
        x = x.reshape(NGRP_PER_B, TOK, BC, HC, 128)     # [tg, t, b, hc, p]
        x = x.transpose(2, 0, 4, 3, 1)                  # [b, tg, p, hc, t]
        return np.ascontiguousarray(x.reshape(NGRP, 128, HC * TOK))

    hid0 = hidden.reshape(B, H)
    in_maps = []
    for c in range(NCORES):
        hc_slice = hid0[c * BC : (c + 1) * BC, :]       # (BC, H)
        hidT = np.ascontiguousarray(
            hc_slice.T.reshape(HC, 128, BC).transpose(1, 0, 2)
            .reshape(128, -1).astype(ml_dtypes.bfloat16)
        )
        in_maps.append(
            {
                "encP": encP_core(c),
                "hidT": hidT,
                "WhT": whT,
                "WeT8": weT8,
                "WeTb": weTb,
                "bsum": bsum_sb,
                "vpg": vpg,
            }
        )
    return in_maps


def kernel(hidden, encoder_outputs, W_h, b_h, W_e, b_e, v):
    nc = _get_nc()
    in_maps = make_in_maps(hidden, encoder_outputs, W_h, b_h, W_e, b_e, v)
    res = run_bass_kernel_spmd(nc, in_maps, list(range(NCORES)))
    full = np.concatenate([res.results[c]["out"] for c in range(NCORES)], axis=0)
    return full[:, None, :].astype(np.float32)


# revision 5
# speedup vs baseline: 1.4113x; 1.1079x over previous
"""Bahdanau attention on 8 Trainium2 cores (Bass/Tile), data-parallel over B.

reference (per batch b, all shapes full):
    hp  = hidden[0] @ W_h.T + b_h                    # (B, H)
    ep  = einsum('tbh,gh->btg', enc, W_e) + b_e      # (B, T, H)
    en  = tanh(hp[:, None, :] + ep)                  # (B, T, H)
    sc  = en @ v                                     # (B, T)
    out = softmax(sc, -1)[:, None, :]                # (B, 1, T)

Sharding: B=32 split 4-per-core across 8 cores; weights replicated.

Design (HW-measured ~82us per main-loop pass vs ~283us for the previous
bf16/XBAR-transpose kernel):

  - Mixed-precision ep matmul keyed on |v|: the scores error contributed
    by output column g is weighted by |v_g| (scores = v . tanh(...)), so
    the host permutes the g axis (rows of W_e/W_h, entries of b_h/b_e/v)
    by DESCENDING |v_g|.  The top 384 columns (~85% of sum v^2) run in
    bf16; the low 640 columns run as fp8-e4m3 DoubleRow matmuls (2
    weights per PE cell, 2x MACs/cycle).  Measured rel err 1.40e-2 vs
    the 2e-2 gate (full-fp8 measures 3.4e-2 -- fails).
  - All weights are host-pretransposed/quantized/permuted into the exact
    SBUF [h_lo, hc, g] layouts (pure data marshalling, zero device work):
    W_e^T fp8 x256, W_e^T bf16, W_h^T bf16, b_h+b_e and v pre-chunked.
  - enc is host-pretransposed per (batch, 512-token group) into the
    moving-tensor layout [h_lo(128), hc, t], so the kernel needs NO
    on-device transpose at all: the old f32->bf16-convert -> XBAR
    -> fp8-cast chain serialized on the DMA fabric and gated every
    group (the cost-model trace showed the pipeline locked to a
    15.5us/group DMA+dependency cycle; eliminating the XBAR collapsed
    it to ~5.2us/group).
  - Per 512-token group: one 2MB DMA (SP queue) loads enc f32; ACT
    converts f32->bf16 with the x16 enc scale folded in; DVE casts
    bf16->fp8 (x256 W scale pre-applied on host; tanh divides out
    16*256=4096 via its scale operand).  PE runs 3 bf16 chunks (8
    matmuls each), the deferred ones-matmul score reduction for the
    previous group (tile_position lands batch b's scores on partition
    32b), then 5 fp8-DoubleRow chunks (4 matmuls each, K=256).
  - tanh on ACT (PSUM f32 -> bf16) with per-partition bias hp^T+b_h+b_e;
    the v-weighted reduction over g runs as 8 chained per-chunk FMAs on
    DVE (scalar_tensor_tensor, f32 accumulator, one final bf16 round).
  - Prefetch tail per iteration: ACT convert for g+2, DVE cast for g+1,
    enc DMA for g+3 at the top -- each a full iteration ahead of its
    consumer so the (scheduler-reordered) in-order engine streams never
    head-of-line block the compute chain.
  - Engine placement keeps every queue under the PE time: enc DMA on the
    otherwise-idle SP queue (NOT the ACT queue -- a DMA occupies the
    issuing queue for the whole transfer), out DMA on Pool, scores copy
    on DVE, and the f32->bf16 convert split 6/2 between ACT and DVE to
    balance the two near-critical engines.  GPSIMD cannot touch PSUM and
    is ~10x software-slow for dtype-converting tensor ops, so it only
    does memset + out-DMAs.  fp32r moving operands measured 2x slower
    than bf16 on HW (rejected).
  - softmax skips the max-subtraction (scores are bounded v.tanh sums,
    so f32 exp is exact-safe) and runs once per pass for all batches.
"""

import sys
from contextlib import ExitStack

import numpy as np
import ml_dtypes

try:
    import concourse  # noqa: F401
except ImportError:  # pragma: no cover
    sys.path.insert(0, "/opt/trn_rl_repo")

import concourse.tile as tile
from concourse import bacc, mybir
from concourse.bass import ts
from concourse.bass_utils import run_bass_kernel_spmd

H = 1024
T = 2048
B = 32
NCORES = 8
BC = B // NCORES          # batches per core
HC = H // 128             # h chunks (contraction)
GC = H // 128             # g chunks (output, |v|-sorted descending)
KF8 = 5                   # low-|v| g chunks computed in fp8 DoubleRow
KBF = GC - KF8            # bf16 chunks (top |v|), run FIRST in each group
G8 = KF8 * 128            # fp8 columns
GBF = H - G8              # bf16 columns
TOK = 512                 # tokens per group (one batch each)
NGRP_PER_B = T // TOK
NGRP = BC * NGRP_PER_B
ESCL = 16.0               # enc scale into bf16/fp8 (keeps e4m3 out of denormals)
WSCL = 256.0              # W_e fp8 scale (W ~ uniform(-1/32,1/32))

F32 = mybir.dt.float32
BF16 = mybir.dt.bfloat16
F8E4 = mybir.dt.float8e4
AF = mybir.ActivationFunctionType
ALU = mybir.AluOpType
PM = mybir.MatmulPerfMode


def build_kernel_nc(reps=1):
    nc = bacc.Bacc(
        "TRN2",
        target_bir_lowering=False,
        debug=False,
        enable_asserts=False,
        num_devices=NCORES,
    )
    enc = nc.dram_tensor("encP", [NGRP, 128, HC * TOK], F32, kind="ExternalInput").ap()
    hidT = nc.dram_tensor("hidT", [128, HC * BC], BF16, kind="ExternalInput").ap()
    whT = nc.dram_tensor("WhT", [128, HC * H], BF16, kind="ExternalInput").ap()
    weT8 = nc.dram_tensor("WeT8", [128, HC * G8], F8E4, kind="ExternalInput").ap()
    weTb = nc.dram_tensor("WeTb", [128, HC * GBF], BF16, kind="ExternalInput").ap()
    bsum = nc.dram_tensor("bsum", [128, GC], F32, kind="ExternalInput").ap()
    vpg = nc.dram_tensor("vpg", [128, GC], F32, kind="ExternalInput").ap()
    out = nc.dram_tensor("out", [BC, T], F32, kind="ExternalOutput").ap()

    with tile.TileContext(nc) as tc:
        _kernel_body(tc, enc, hidT, whT, weT8, weTb, bsum, vpg, out, reps=reps)
    nc.compile()
    return nc


def _kernel_body(tc, enc, hidT_d, whT_d, weT8_d, weTb_d, bsum_d, vpg_d, out, reps=1):
    nc = tc.nc
    with ExitStack() as ctx:
        singles = ctx.enter_context(tc.tile_pool(name="singles", bufs=1))
        enc_pool = ctx.enter_context(tc.tile_pool(name="enc_f32", bufs=3))
        encT_pool = ctx.enter_context(tc.tile_pool(name="encT", bufs=3))
        encT8_pool = ctx.enter_context(tc.tile_pool(name="encT8", bufs=3))
        energy_pool = ctx.enter_context(tc.tile_pool(name="energy", bufs=6))
        zf_pool = ctx.enter_context(tc.tile_pool(name="zf32", bufs=2))
        zb_pool = ctx.enter_context(tc.tile_pool(name="zb16", bufs=3))
        ep_pool = ctx.enter_context(tc.tile_pool(name="epps", bufs=5, space="PSUM"))
        sc_pool = ctx.enter_context(tc.tile_pool(name="scps", bufs=2, space="PSUM"))

        # ---- persistent SBUF tensors -------------------------------------
        WeT8 = singles.tile([128, HC // 2, KF8, 2, 128], F8E4)  # SwInterleave blocks
        WeTb = singles.tile([128, HC, GBF], BF16)  # [h_lo, hc, g']
        bias_all = singles.tile([128, GC, BC], F32)  # hp^T + b_h + b_e (per g')
        v_sb = singles.tile([128, GC], F32)
        ones_bf = singles.tile([128, 1], BF16)
        bsum_sb = singles.tile([128, GC], F32)
        scores = singles.tile([128, T], F32)       # batch b at partition 32*b
        probs = singles.tile([128, T], F32)
        sums = singles.tile([128, 1], F32)
        rsum = singles.tile([128, 1], F32)

        # ---- stage 0: weight DMAs (host-pretransposed) + hp --------------
        nc.sync.dma_start(out=WeT8[:], in_=weT8_d.rearrange("p (j g i m) -> p j g i m", j=HC // 2, g=KF8, i=2))
        nc.sync.dma_start(out=WeTb[:], in_=weTb_d.rearrange("p (c g) -> p c g", c=HC))
        nc.sync.dma_start(out=bsum_sb[:], in_=bsum_d[:, :])
        nc.sync.dma_start(out=v_sb[:], in_=vpg_d[:, :])
        nc.gpsimd.memset(ones_bf[:], 1.0)

        with tc.tile_pool(name="stage0", bufs=1) as wstat, tc.tile_pool(
            name="hpps", bufs=1, space="PSUM"
        ) as hp_pool:
            WhT = wstat.tile([128, HC, H], BF16, tag="wht")
            hidT = wstat.tile([128, HC, BC], BF16, tag="hidt")
            nc.sync.dma_start(out=WhT[:], in_=whT_d.rearrange("p (c g) -> p c g", c=HC))
            nc.sync.dma_start(out=hidT[:], in_=hidT_d.rearrange("p (c b) -> p c b", c=HC))
            for gc in range(GC):
                hp_ps = hp_pool.tile([128, BC], F32, tag="hp")
                for hc in range(HC):
                    nc.tensor.matmul(
                        hp_ps[:],
                        WhT[:, hc, ts(gc, 128)],
                        hidT[:, hc, :],
                        start=(hc == 0),
                        stop=(hc == HC - 1),
                    )
                nc.vector.tensor_scalar(
                    out=bias_all[:, gc, :],
                    in0=hp_ps[:],
                    scalar1=bsum_sb[:, gc : gc + 1],
                    scalar2=None,
                    op0=ALU.add,
                )

        # ---- main loop: 16 groups of 512 tokens --------------------------
        n_total = reps * NGRP

        def issue_load(grp):
            g = grp % NGRP
            en = enc_pool.tile([128, HC, TOK], F32, tag="en")
            nc.sync.dma_start(
                out=en[:], in_=enc[g, :, :].rearrange("p (c t) -> p c t", c=HC)
            )
            return en

        def issue_convert(en_f32):
            # f32 -> bf16 with the x16 enc scale folded in; split 6/2 over
            # ACT and DVE to balance the two near-critical engines.  The
            # host already staged enc in the transposed [h_lo, hc, t] layout.
            encT = encT_pool.tile([128, HC, TOK], BF16)
            nc.scalar.activation(
                out=encT[:, 0:6, :], in_=en_f32[:, 0:6, :],
                func=AF.Copy, bias=0.0, scale=ESCL,
            )
            nc.vector.tensor_scalar_mul(
                encT[:, 6:HC, :], en_f32[:, 6:HC, :], ESCL
            )
            return encT

        def issue_f8cast(encT):
            # pure dtype cast bf16 -> fp8 (enc already x16-scaled) on DVE
            encT8 = encT8_pool.tile([128, HC, TOK], F8E4)
            nc.vector.tensor_copy(encT8[:], encT[:])
            return encT8

        loads = {g: issue_load(g) for g in range(3)}
        encTs = {0: issue_convert(loads[0]), 1: issue_convert(loads[1])}
        encT8s = {0: issue_f8cast(encTs[0])}
        carry = None  # deferred z reduction of the previous group

        def softmax_b(b):
            r = slice(32 * b, 32 * b + 1)
            # |scores| is bounded (sum of v*tanh terms), so exp without the
            # max subtraction is safe in f32 and matches softmax exactly.
            nc.scalar.activation(
                out=probs[r, :], in_=scores[r, :], func=AF.Exp,
                bias=0.0, scale=1.0, accum_out=sums[r],
            )
            nc.vector.reciprocal(out=rsum[r], in_=sums[r])
            nc.vector.tensor_scalar_mul(probs[r, :], probs[r, :], rsum[r])
            nc.gpsimd.dma_start(out=out[b : b + 1, :], in_=probs[r, :])

        def flush_carry(c):
            c_z, c_b, c_t0 = c
            sc_ps = sc_pool.tile([128, TOK], F32)
            r = slice(32 * c_b, 32 * c_b + 1)
            nc.tensor.matmul(
                sc_ps[r, :], ones_bf[:], c_z[:],
                start=True, stop=True, tile_position=(0, 32 * c_b),
            )
            nc.vector.tensor_copy(scores[r, c_t0 : c_t0 + TOK], sc_ps[r, :])

        for grp in range(n_total):
            g = grp % NGRP
            b = g // NGRP_PER_B
            t0 = (g % NGRP_PER_B) * TOK

            if grp + 3 < n_total:
                loads[grp + 3] = issue_load(grp + 3)

            encT_cur = encTs.pop(grp)
            encT8_cur = encT8s.pop(grp)
            for gc in range(GC):
                ep_ps = ep_pool.tile([128, TOK], F32)
                if gc >= KBF:
                    for j in range(HC // 2):
                        nc.tensor.matmul(
                            ep_ps[:],
                            WeT8[:, j, gc - KBF, :, :],
                            encT8_cur[:, 2 * j : 2 * j + 2, :],
                            start=(j == 0),
                            stop=(j == HC // 2 - 1),
                            perf_mode=PM.DoubleRowSwInterleave,
                        )
                    tanh_scale = 1.0 / (ESCL * WSCL)
                else:
                    for hc in range(HC):
                        nc.tensor.matmul(
                            ep_ps[:],
                            WeTb[:, hc, ts(gc, 128)],
                            encT_cur[:, hc, :],
                            start=(hc == 0),
                            stop=(hc == HC - 1),
                        )
                    tanh_scale = 1.0 / ESCL
                if gc == KBF and carry is not None:
                    # flush at the bf16->fp8 boundary (no extra mode switch)
                    flush_carry(carry)
                    carry = None
                e_ch = energy_pool.tile([128, TOK], BF16)
                nc.scalar.activation(
                    out=e_ch[:],
                    in_=ep_ps[:],
                    func=AF.Tanh,
                    bias=bias_all[:, gc, b : b + 1],
                    scale=tanh_scale,
                )
                # z += v_chunk * energy_chunk, f32 accumulation on DVE; the
                # last chunk rounds once to bf16 for the ones-matmul.
                if gc == 0:
                    zf = zf_pool.tile([128, TOK], F32)
                    nc.vector.tensor_scalar_mul(zf[:], e_ch[:], v_sb[:, 0:1])
                elif gc < GC - 1:
                    zf_new = zf_pool.tile([128, TOK], F32)
                    nc.vector.scalar_tensor_tensor(
                        out=zf_new[:], in0=e_ch[:],
                        scalar=v_sb[:, gc : gc + 1], in1=zf[:],
                        op0=ALU.mult, op1=ALU.add,
                    )
                    zf = zf_new
                else:
                    zb = zb_pool.tile([128, TOK], BF16)
                    nc.vector.scalar_tensor_tensor(
                        out=zb[:], in0=e_ch[:],
                        scalar=v_sb[:, gc : gc + 1], in1=zf[:],
                        op0=ALU.mult, op1=ALU.add,
                    )
            carry = (zb, b, t0)
            # prefetch tail: ACT convert for g+2, DVE fp8 cast for g+1 —
            # each a full iteration ahead of its consumer, so even
            # scheduler-reordered they never block the compute chain.
            if grp + 2 < n_total:
                encTs[grp + 2] = issue_convert(loads.pop(grp + 2))
            if grp + 1 < n_total:
                encT8s[grp + 1] = issue_f8cast(encTs[grp + 1])

            if g == NGRP - 1:
                flush_carry(carry)
                carry = None
                for bb in range(BC):
                    softmax_b(bb)


_NC_CACHE = None


def _get_nc():
    global _NC_CACHE
    if _NC_CACHE is None:
        _NC_CACHE = build_kernel_nc()
    return _NC_CACHE


E4M3 = ml_dtypes.float8_e4m3


def make_in_maps(hidden, encoder_outputs, W_h, b_h, W_e, b_e, v):
    """Host-side marshalling: shard over B, |v|-sort the g axis, transpose
    weights and enc into the SBUF layouts, quantize the fp8 weight part."""
    hidden = np.asarray(hidden, dtype=np.float32)
    enc = np.asarray(encoder_outputs, dtype=np.float32)
    W_h = np.asarray(W_h, dtype=np.float32)
    W_e = np.asarray(W_e, dtype=np.float32)
    b_h = np.asarray(b_h, dtype=np.float32)
    b_e = np.asarray(b_e, dtype=np.float32)
    v = np.asarray(v, dtype=np.float32)

    perm = np.argsort(np.abs(v), kind="stable")[::-1]   # descending |v|
    Wep = W_e[perm]
    Whp = W_h[perm]
    bsum_p = (b_h + b_e)[perm]
    vp = v[perm]

    def to_sb(mat_T, dtype):             # mat_T: (H_in, G) = W^T
        a = mat_T.reshape(HC, 128, -1).transpose(1, 0, 2)  # (128, HC, G)
        return np.ascontiguousarray(
            a.reshape(128, -1).astype(dtype, copy=False)
        )

    WeT = Wep.T                          # (h, g'), |v| descending over g'
    weTb = to_sb(np.asarray(WeT[:, :GBF], dtype=ml_dtypes.bfloat16),
                 ml_dtypes.bfloat16)
    # fp8 stationary blocks pre-interleaved for DoubleRowSwInterleave:
    # per (j, g-chunk) block, flat[p, 2*(127-m)+i] = W'[g0+m, (2j+i)*128+p]
    w8 = np.asarray(WeT[:, GBF:] * WSCL, dtype=E4M3)     # (H, G8)
    w8 = w8.reshape(HC // 2, 2, 128, KF8, 128)           # [j, i, p, gc, m]
    ilv = np.empty((128, HC // 2, KF8, 256), dtype=E4M3) # [p, j, gc, k]
    for m in range(128):
        for i in range(2):
            ilv[:, :, :, 2 * (127 - m) + i] = w8[:, i, :, :, m].transpose(1, 0, 2)
    weT8 = np.ascontiguousarray(ilv.reshape(128, -1))
    whT = to_sb(np.asarray(Whp.T, dtype=ml_dtypes.bfloat16), ml_dtypes.bfloat16)
    bsum_sb = np.ascontiguousarray(bsum_p.reshape(GC, 128).T)
    vpg = np.ascontiguousarray(vp.reshape(GC, 128).T)

    def encP_core(c):
        x = enc[:, c * BC : (c + 1) * BC, :]            # (T, BC, H)
        x = x.reshape(NGRP_PER_B, TOK, BC, HC, 128)     # [tg, t, b, hc, p]
        x = x.transpose(2, 0, 4, 3, 1)                  # [b, tg, p, hc, t]
        return np.ascontiguousarray(x.reshape(NGRP, 128, HC * TOK))

    hid0 = hidden.reshape(B, H)
    in_maps = []
    for c in range(NCORES):
        hc_slice = hid0[c * BC : (c + 1) * BC, :]       # (BC, H)
        hidT = np.ascontiguousarray(
            hc_slice.T.reshape(HC, 128, BC).transpose(1, 0, 2)
            .reshape(128, -1).astype(ml_dtypes.bfloat16)
        )
        in_maps.append(
            {
                "encP": encP_core(c),
                "hidT": hidT,
                "WhT": whT,
                "WeT8": weT8,
                "WeTb": weTb,
                "bsum": bsum_sb,
                "vpg": vpg,
            }
        )
    return in_maps


def kernel(hidden, encoder_outputs, W_h, b_h, W_e, b_e, v):
    nc = _get_nc()
    in_maps = make_in_maps(hidden, encoder_outputs, W_h, b_h, W_e, b_e, v)
    res = run_bass_kernel_spmd(nc, in_maps, list(range(NCORES)))
    full = np.concatenate([res.results[c]["out"] for c in range(NCORES)], axis=0)
    return full[:, None, :].astype(np.float32)
